# revision 3
# baseline (speedup 1.0000x reference)
"""Multi-head attention with LoRA adapters on 8 Trainium2 NeuronCores.

Problem: x[4,2048,768] -> LoRA-linear QKV -> 12-head attention -> LoRA-linear out proj.

Math notes:
  - LoRA is folded into the base weights on the host:  x@W.T + b + (x@A.T)@B.T
    == x@(W + B@A).T + b  (exact up to fp rounding).
  - The value bias bv is folded into the output bias using softmax(row)@1 == 1:
    (attn@(v + bv)) @ Wo.T + bo == (attn@v)@Wo.T + (bo + Wo@bv).
  - Softmax is computed without max-subtraction (scores are O(+-10), safe in fp32);
    the row sum comes from a dedicated ones-column matmul pair, and the division
    is applied to the (tiny) attention output rather than the attention matrix.

Sharding: core = 2*b + g for batch b in 0..3, head-group g in 0..1 (6 heads each).
Each core computes its 6 heads' attention output and a partial output projection
(row-sharded Wo); host sums the two partials per batch.

Per-core structure (DIM=768, CS=384 local channels, T=2048), heads processed as
3 PAIRS (2 heads per 128-channel block):
  scores: S^T pair = row-tiled matmul pair (K=64 each, tile rows 0-63 / 64-127)
          -> one [128,1024] f32 PSUM tile (two banks, one per head).
  exp:    ONE scalar-engine activation per (pair, j, s) step reads the fused
          [128,1024] PSUM tile -> e bf16 SBUF.  The ACT engine is the kernel
          bottleneck (T*T*6 exp elements at 1 elem/cycle/lane) so everything
          else is scheduled around keeping it saturated.
  PV:     col-tiled matmul pair (M=64 at array cols 0-63 / 64-127) accumulating
          into ONE [128,512] PSUM bank + a concurrent ones-pair (M=1 at cols
          0 / 32) accumulating row sums into a [33,512] bank.
  norm:   reciprocal of the sums, partition-broadcast via a DRAM bounce,
          one [128,512] multiply writes the pair block of outT directly.

Schedule: PV runs one j-chunk (16 s-steps) behind the scores so the in-order
PE queue never waits on the scalar engine; QKV/V projections and the output
projection are injected into the PE slack between score matmuls.
"""

import contextlib
import os
import sys

sys.path.insert(0, "/opt/trn_rl_repo")

import numpy as np

# Wrap the whole kernel body in a hardware loop (timing harness only; the
# graded path uses REPEAT=1).
REPEAT = int(os.environ.get("KERNEL_REPEAT", "1"))

DIM, HEADS, R = 768, 12, 8
B, T = 4, 2048
HD = DIM // HEADS          # 64 head dim
NCORES = 8
HG = HEADS // 2            # 6 heads per core
CS = HG * HD               # 384 local channels per core
SCALE = HD ** -0.5

_PROGRAM_CACHE = {}


def _bf16(a):
    import ml_dtypes
    return np.ascontiguousarray(a).astype(ml_dtypes.bfloat16)


def _build_program():
    import concourse.bass as bass
    import concourse.mybir as mybir
    import concourse.tile as tile
    from concourse import bacc

    f32 = mybir.dt.float32
    bf16 = mybir.dt.bfloat16

    nc = bacc.Bacc("TRN2", target_bir_lowering=False, debug=False,
                   num_devices=NCORES)

    xT = nc.dram_tensor("xT", [DIM, T], bf16, kind="ExternalInput")
    wq_t = nc.dram_tensor("wq_t", [DIM, CS], bf16, kind="ExternalInput")
    wk_t = nc.dram_tensor("wk_t", [DIM, CS], bf16, kind="ExternalInput")
    wv_t = nc.dram_tensor("wv_t", [DIM, CS], bf16, kind="ExternalInput")
    wo_t = nc.dram_tensor("wo_t", [CS, DIM], bf16, kind="ExternalInput")
    bq_s = nc.dram_tensor("bq_s", [CS], f32, kind="ExternalInput")
    bk_s = nc.dram_tensor("bk_s", [CS], f32, kind="ExternalInput")
    bo_s = nc.dram_tensor("bo_s", [DIM], f32, kind="ExternalInput")
    out_p = nc.dram_tensor("out_p", [T, DIM], f32, kind="ExternalOutput")

    KB = DIM // 128      # 6 k-blocks of the input dim
    CB = CS // 128       # 3 channel blocks (head pairs)
    TB = T // 128        # 16 s-steps (key/value time blocks)
    JB = T // 512        # 4 j-chunks (query time columns)
    NCHUNK = CB * JB     # 12 (pair, j) chunks

    with tile.TileContext(nc) as tc:
        with (
            tc.tile_pool(name="weights", bufs=1) as wpool,
            tc.tile_pool(name="persist", bufs=1) as ppool,
            tc.tile_pool(name="ps", bufs=2, space="PSUM") as ps_pool,
            tc.tile_pool(name="psC", bufs=2, space="PSUM") as psC_pool,
            tc.tile_pool(name="psS", bufs=2, space="PSUM") as psS_pool,
            tc.tile_pool(name="epool", bufs=20) as e_pool,
            tc.tile_pool(name="npool", bufs=2) as n_pool,
            tc.tile_pool(name="opool", bufs=2) as o_pool,
            tc.tile_pool(name="rspool", bufs=4, space="DRAM") as rs_pool,
            (tc.For_i(0, REPEAT, 1) if REPEAT > 1 else contextlib.nullcontext()),
        ):
            # ---- input loads (w_ot/bo deferred until late) ----
            w_kt = wpool.tile([128, KB, CS], bf16)
            nc.sync.dma_start(out=w_kt, in_=wk_t.ap().rearrange("(k p) m -> p k m", p=128))
            w_qt = wpool.tile([128, KB, CS], bf16)
            nc.sync.dma_start(out=w_qt, in_=wq_t.ap().rearrange("(k p) m -> p k m", p=128))
            xT_sb = wpool.tile([128, KB, T], bf16)
            xT_view = xT.ap().rearrange("(k p) t -> p k t", p=128)
            for kk in range(KB):
                nc.sync.dma_start(out=xT_sb[:, kk, :], in_=xT_view[:, kk, :])
            w_vt = wpool.tile([128, KB, CS], bf16)
            nc.sync.dma_start(out=w_vt, in_=wv_t.ap().rearrange("(k p) m -> p k m", p=128))
            bq_sb = wpool.tile([128, CB], f32)
            nc.sync.dma_start(out=bq_sb, in_=bq_s.ap().rearrange("(k p) -> p k", p=128))
            bk_sb = wpool.tile([128, CB], f32)
            nc.sync.dma_start(out=bk_sb, in_=bk_s.ap().rearrange("(k p) -> p k", p=128))

            # ---- persistent activations ----
            qT_sb = ppool.tile([128, CB, T], bf16)
            kT_sb = ppool.tile([128, CB, T], bf16)
            v_sb = ppool.tile([128, TB, CS], bf16)
            outT_sb = ppool.tile([128, CB, T], bf16)
            ones_col = ppool.tile([128, 1], bf16)
            nc.vector.memset(ones_col, 1.0)
            w_ot = wpool.tile([128, CB, DIM], bf16)
            bo_sb = wpool.tile([128, DIM], f32)

            def proj_qk_thunks(dst, w, bias, cb):
                # one channel block of the q^T / k^T projection, 4 thunks
                def mk(jt):
                    def thunk():
                        psp = ps_pool.tile([128, 1024], f32, tag="ps")
                        for kk in range(KB):
                            nc.tensor.matmul(
                                psp[:, 0:512],
                                w[:, kk, cb * 128:(cb + 1) * 128],
                                xT_sb[:, kk, jt * 512:(jt + 1) * 512],
                                start=(kk == 0), stop=(kk == KB - 1),
                            )
                        nc.vector.tensor_scalar_add(
                            dst[:, cb, jt * 512:(jt + 1) * 512], psp[:, 0:512],
                            bias[:, cb:cb + 1],
                        )
                    return thunk
                return [mk(jt) for jt in range(JB)]

            def proj_v_thunks():
                # v[t, c] projection, one st-tile per thunk
                def mk(st):
                    def thunk():
                        psp = psC_pool.tile([128, 512], f32, tag="psC")
                        for kk in range(KB):
                            nc.tensor.matmul(
                                psp[:, 0:CS],
                                xT_sb[:, kk, st * 128:(st + 1) * 128],
                                w_vt[:, kk, :],
                                start=(kk == 0), stop=(kk == KB - 1),
                            )
                        nc.vector.tensor_copy(v_sb[:, st, :], psp[:, 0:CS])
                    return thunk
                return [mk(st) for st in range(TB)]

            def load_tail_thunk():
                def thunk():
                    nc.sync.dma_start(
                        out=w_ot, in_=wo_t.ap().rearrange("(k p) m -> p k m", p=128))
                    bo_row = wpool.tile([1, DIM], f32)
                    nc.sync.dma_start(out=bo_row, in_=bo_s.ap().rearrange("(o d) -> o d", o=1))
                    nc.gpsimd.partition_broadcast(bo_sb, bo_row)
                return [thunk]

            def out_proj_thunks(mts):
                # output projection tiles; each needs outT for all 3 pairs in
                # its mt window (i.e. norms (p, mt//4) for all p emitted).
                def mk(mt):
                    def thunk():
                        psD = ps_pool.tile([128, 1024], f32, tag="ps")
                        for ncols in (slice(0, 512), slice(512, DIM)):
                            for cb in range(CB):
                                nc.tensor.matmul(
                                    psD[:, ncols],
                                    outT_sb[:, cb, mt * 128:(mt + 1) * 128],
                                    w_ot[:, cb, ncols],
                                    start=(cb == 0), stop=(cb == CB - 1),
                                )
                        osb = o_pool.tile([128, DIM], f32, tag="osb")
                        nc.vector.tensor_add(osb, psD[:, 0:DIM], bo_sb)
                        nc.sync.dma_start(
                            out=out_p.ap()[mt * 128:(mt + 1) * 128, :], in_=osb,
                        )
                    return thunk
                return [mk(mt) for mt in mts]

            # ---- per-step emitters ----
            e_tiles = {}     # (chunk, s) -> e tile (live for one j-chunk lag)
            psC_cur = {}     # chunk -> psC tile
            psS_cur = {}     # chunk -> psSum tile

            def emit_scores(c, s):
                pair, j = divmod(c, JB)
                jcols = slice(j * 512, (j + 1) * 512)
                psB = ps_pool.tile([128, 1024], f32, tag="ps")
                nc.tensor.matmul(
                    psB[:, 0:512],
                    kT_sb[0:64, pair, s * 128:(s + 1) * 128],
                    qT_sb[0:64, pair, jcols],
                    start=True, stop=True,
                )
                nc.tensor.matmul(
                    psB[:, 512:1024],
                    kT_sb[64:128, pair, s * 128:(s + 1) * 128],
                    qT_sb[64:128, pair, jcols],
                    start=True, stop=True,
                )
                e = e_pool.tile([128, 1024], bf16, tag="e")
                nc.scalar.activation(
                    e, psB, mybir.ActivationFunctionType.Exp, scale=SCALE,
                )
                e_tiles[(c, s)] = e

            def emit_pv(c, s):
                pair, j = divmod(c, JB)
                if s == 0:
                    psC_cur[c] = psC_pool.tile([128, 512], f32, tag="psC", name=f"psC{c}")
                    psS_cur[c] = psS_pool.tile([33, 512], f32, tag="psS", name=f"psS{c}")
                psC = psC_cur[c]
                psS = psS_cur[c]
                e = e_tiles.pop((c, s))
                first, last = (s == 0), (s == TB - 1)
                for hb in range(2):
                    h = 2 * pair + hb
                    ecols = slice(hb * 512, hb * 512 + 512)
                    nc.tensor.matmul(
                        psC[hb * 64:(hb + 1) * 64, :],
                        v_sb[:, s, h * HD:(h + 1) * HD],
                        e[:, ecols],
                        start=first, stop=last,
                    )
                for hb in range(2):
                    ecols = slice(hb * 512, hb * 512 + 512)
                    nc.tensor.matmul(
                        psS[hb * 32:hb * 32 + 1, :],
                        ones_col,
                        e[:, ecols],
                        start=first, stop=last,
                    )

            def emit_norm(c):
                pair, j = divmod(c, JB)
                jcols = slice(j * 512, (j + 1) * 512)
                psC = psC_cur.pop(c)
                psS = psS_cur.pop(c)
                nr = n_pool.tile([33, 512], f32, tag="nr")
                nc.vector.reciprocal(nr[0:1, :], psS[0:1, :])
                nc.vector.reciprocal(nr[32:33, :], psS[32:33, :])
                # partition-broadcast of the two recip rows via a DRAM bounce
                # (gpsimd partition_broadcast reads the wrong partition for
                # base!=0 sources on HW)
                rsA = rs_pool.tile([1, 512], f32, tag="rs")
                nc.sync.dma_start(out=rsA, in_=nr[0:1, :])
                rsB = rs_pool.tile([1, 512], f32, tag="rs")
                nc.sync.dma_start(out=rsB, in_=nr[32:33, :])
                nrbc = n_pool.tile([128, 512], f32, tag="nrbc")
                bcA = bass.AP(tensor=rsA.tensor, offset=rsA.offset,
                              ap=[[0, 64], [1, 512]])
                bcB = bass.AP(tensor=rsB.tensor, offset=rsB.offset,
                              ap=[[0, 64], [1, 512]])
                nc.sync.dma_start(out=nrbc[0:64, :], in_=bcA)
                nc.sync.dma_start(out=nrbc[64:128, :], in_=bcB)
                nc.vector.tensor_mul(outT_sb[:, pair, jcols], psC, nrbc)

            # ---- prologue: q/k projections for pair 0 ----
            for t in proj_qk_thunks(kT_sb, w_kt, bk_sb, 0):
                t()
            for t in proj_qk_thunks(qT_sb, w_qt, bq_sb, 0):
                t()

            # ---- injection lists per chunk ----
            inject = {c: [] for c in range(NCHUNK)}
            inject[0] = proj_v_thunks()
            inject[1] = (proj_qk_thunks(kT_sb, w_kt, bk_sb, 1)
                         + proj_qk_thunks(qT_sb, w_qt, bq_sb, 1))
            inject[4] = (proj_qk_thunks(kT_sb, w_kt, bk_sb, 2)
                         + proj_qk_thunks(qT_sb, w_qt, bq_sb, 2)
                         + load_tail_thunk())
            inject[10] = out_proj_thunks(range(0, 4))
            inject[11] = out_proj_thunks(range(4, 8))

            # ---- main loop: scores for chunk c, PV lagged one chunk ----
            for c in range(NCHUNK):
                pending = list(inject[c])
                for s in range(TB):
                    emit_scores(c, s)
                    if pending:
                        pending.pop(0)()
                    if c > 0:
                        emit_pv(c - 1, s)
                if c > 0:
                    emit_norm(c - 1)
                for t in pending:
                    t()

            # ---- tail: last chunk's PV + remaining output projection ----
            tail_inject = out_proj_thunks(range(8, 12))
            for s in range(TB):
                emit_pv(NCHUNK - 1, s)
                if s % 4 == 3:
                    tail_inject.pop(0)()
            emit_norm(NCHUNK - 1)
            for t in out_proj_thunks(range(12, 16)):
                t()

    nc.compile()
    return nc


def _get_program():
    if "nc" not in _PROGRAM_CACHE:
        _PROGRAM_CACHE["nc"] = _build_program()
    return _PROGRAM_CACHE["nc"]


def _prep_in_maps(inputs):
    f = np.float32

    def eff(w, a, bl):
        return (w.astype(np.float64) + bl.astype(np.float64) @ a.astype(np.float64)).astype(f)

    wq = eff(inputs["wq"], inputs["laq"], inputs["lbq"])
    wk = eff(inputs["wk"], inputs["lak"], inputs["lbk"])
    wv = eff(inputs["wv"], inputs["lav"], inputs["lbv"])
    wo = eff(inputs["wo"], inputs["lao"], inputs["lbo"])
    x = np.asarray(inputs["x"], dtype=f)
    bq, bk, bv, bo = (np.asarray(inputs[k], dtype=f) for k in ("bq", "bk", "bv", "bo"))

    in_maps = []
    for core in range(NCORES):
        b, g = core // 2, core % 2
        cols = slice(g * CS, (g + 1) * CS)
        bo_core = wo[:, cols].astype(np.float64) @ bv[cols].astype(np.float64)
        if g == 0:
            bo_core = bo_core + bo
        in_maps.append({
            "xT": _bf16(x[b].T),
            "wq_t": _bf16(wq[cols, :].T),
            "wk_t": _bf16(wk[cols, :].T),
            "wv_t": _bf16(wv[cols, :].T),
            "wo_t": _bf16(wo[:, cols].T),
            "bq_s": np.ascontiguousarray(bq[cols]),
            "bk_s": np.ascontiguousarray(bk[cols]),
            "bo_s": bo_core.astype(f),
        })
    return in_maps


def kernel(**inputs):
    from concourse.bass_utils import run_bass_kernel_spmd

    nc = _get_program()
    in_maps = _prep_in_maps(inputs)
    res = run_bass_kernel_spmd(nc, in_maps, core_ids=list(range(NCORES)))
    out = np.empty((B, T, DIM), dtype=np.float32)
    for b in range(B):
        out[b] = res.results[2 * b]["out_p"] + res.results[2 * b + 1]["out_p"]
    return out


# revision 6
# speedup vs baseline: 1.4501x; 1.4501x over previous
"""Multi-head attention with LoRA adapters on 8 Trainium2 NeuronCores.

Problem: x[4,2048,768] -> LoRA-linear QKV -> 12-head attention -> LoRA-linear out proj.

Math notes:
  - LoRA is folded into the base weights on the host:  x@W.T + b + (x@A.T)@B.T
    == x@(W + B@A).T + b  (exact up to fp rounding).
  - The value bias bv is folded into the output bias using softmax(row)@1 == 1:
    (attn@(v + bv)) @ Wo.T + bo == (attn@v)@Wo.T + (bo + Wo@bv).
  - Softmax is computed without max-subtraction (scores are O(+-10), safe in fp32);
    the row sum comes from a dedicated ones-column matmul pair, and the division
    is applied to the (tiny) attention output rather than the attention matrix.

Sharding: core = 2*b + g for batch b in 0..3, head-group g in 0..1 (6 heads each).
Each core computes its 6 heads' attention output and a partial output projection
(row-sharded Wo); host sums the two partials per batch.

Per-core structure (DIM=768, CS=384 local channels, T=2048), heads processed as
3 PAIRS (2 heads per 128-channel block):
  scores: S^T pair = row-tiled matmul pair (K=64 each, tile rows 0-63 / 64-127)
          -> one [128,1024] f32 PSUM tile (two banks, one per head).
  exp:    ONE scalar-engine activation per (pair, j, s) step reads the fused
          [128,1024] PSUM tile -> e bf16 SBUF.  The ACT engine is the kernel
          bottleneck (T*T*6 exp elements at 1 elem/cycle/lane) so everything
          else is scheduled around keeping it saturated.
  PV:     col-tiled matmul pair (M=64 at array cols 0-63 / 64-127) accumulating
          into ONE [128,512] PSUM bank.
  sums:   e tiles are also accumulated elementwise on the Vector engine into a
          bf16 [128,1024] per-chunk accumulator (the PE reads e once, the DVE
          absorbs the second pass); one tiny ones-matmul pair per chunk
          partition-reduces the accumulator into a [33,512] PSUM bank.
  norm:   reciprocal of the sums, partition-broadcast via a DRAM bounce,
          one [128,512] multiply writes the pair block of outT directly.

Schedule: PV runs one j-chunk (16 s-steps) behind the scores so the in-order
PE queue never waits on the scalar engine; QKV/V projections and the output
projection are injected into the PE slack between score matmuls.
"""

import contextlib
import os
import sys

sys.path.insert(0, "/opt/trn_rl_repo")

import numpy as np

# Wrap the whole kernel body in a hardware loop (timing harness only; the
# graded path uses REPEAT=1).
REPEAT = int(os.environ.get("KERNEL_REPEAT", "1"))

DIM, HEADS, R = 768, 12, 8
B, T = 4, 2048
HD = DIM // HEADS          # 64 head dim
NCORES = 8
HG = HEADS // 2            # 6 heads per core
CS = HG * HD               # 384 local channels per core
SCALE = HD ** -0.5

_PROGRAM_CACHE = {}


def _bf16(a):
    import ml_dtypes
    return np.ascontiguousarray(a).astype(ml_dtypes.bfloat16)


def _build_program():
    import concourse.bass as bass
    import concourse.mybir as mybir
    import concourse.tile as tile
    from concourse import bacc

    f32 = mybir.dt.float32
    bf16 = mybir.dt.bfloat16

    nc = bacc.Bacc("TRN2", target_bir_lowering=False, debug=False,
                   num_devices=NCORES)

    xT = nc.dram_tensor("xT", [DIM, T], bf16, kind="ExternalInput")
    wq_t = nc.dram_tensor("wq_t", [DIM, CS], bf16, kind="ExternalInput")
    wk_t = nc.dram_tensor("wk_t", [DIM, CS], bf16, kind="ExternalInput")
    wv_t = nc.dram_tensor("wv_t", [DIM, CS], bf16, kind="ExternalInput")
    wo_t = nc.dram_tensor("wo_t", [CS, DIM], bf16, kind="ExternalInput")
    bq_s = nc.dram_tensor("bq_s", [CS], f32, kind="ExternalInput")
    bk_s = nc.dram_tensor("bk_s", [CS], f32, kind="ExternalInput")
    bo_s = nc.dram_tensor("bo_s", [DIM], f32, kind="ExternalInput")
    out_p = nc.dram_tensor("out_p", [T, DIM], f32, kind="ExternalOutput")

    KB = DIM // 128      # 6 k-blocks of the input dim
    CB = CS // 128       # 3 channel blocks (head pairs)
    TB = T // 128        # 16 s-steps (key/value time blocks)
    JB = T // 512        # 4 j-chunks (query time columns)
    NCHUNK = CB * JB     # 12 (pair, j) chunks

    with tile.TileContext(nc) as tc:
        with (
            tc.tile_pool(name="weights", bufs=1) as wpool,
            tc.tile_pool(name="persist", bufs=1) as ppool,
            tc.tile_pool(name="ps", bufs=2, space="PSUM") as ps_pool,
            tc.tile_pool(name="psC", bufs=2, space="PSUM") as psC_pool,
            tc.tile_pool(name="psS", bufs=2, space="PSUM") as psS_pool,
            tc.tile_pool(name="epool", bufs=20) as e_pool,
            tc.tile_pool(name="npool", bufs=2) as n_pool,
            tc.tile_pool(name="opool", bufs=2) as o_pool,
            tc.tile_pool(name="rspool", bufs=4, space="DRAM") as rs_pool,
            (tc.For_i(0, REPEAT, 1) if REPEAT > 1 else contextlib.nullcontext()),
        ):
            # ---- input loads (w_ot/bo deferred until late) ----
            w_kt = wpool.tile([128, KB, CS], bf16)
            nc.sync.dma_start(out=w_kt, in_=wk_t.ap().rearrange("(k p) m -> p k m", p=128))
            w_qt = wpool.tile([128, KB, CS], bf16)
            nc.sync.dma_start(out=w_qt, in_=wq_t.ap().rearrange("(k p) m -> p k m", p=128))
            xT_sb = wpool.tile([128, KB, T], bf16)
            xT_view = xT.ap().rearrange("(k p) t -> p k t", p=128)
            for kk in range(KB):
                nc.sync.dma_start(out=xT_sb[:, kk, :], in_=xT_view[:, kk, :])
            w_vt = wpool.tile([128, KB, CS], bf16)
            nc.sync.dma_start(out=w_vt, in_=wv_t.ap().rearrange("(k p) m -> p k m", p=128))
            bq_sb = wpool.tile([128, CB], f32)
            nc.sync.dma_start(out=bq_sb, in_=bq_s.ap().rearrange("(k p) -> p k", p=128))
            bk_sb = wpool.tile([128, CB], f32)
            nc.sync.dma_start(out=bk_sb, in_=bk_s.ap().rearrange("(k p) -> p k", p=128))

            # ---- persistent activations ----
            qT_sb = ppool.tile([128, CB, T], bf16)
            kT_sb = ppool.tile([128, CB, T], bf16)
            v_sb = ppool.tile([128, TB, CS], bf16)
            outT_sb = ppool.tile([128, CB, T], bf16)
            ones_col = ppool.tile([128, 1], bf16)
            nc.vector.memset(ones_col, 1.0)
            w_ot = wpool.tile([128, CB, DIM], bf16)
            bo_sb = wpool.tile([128, DIM], f32)

            def proj_qk_thunks(dst, w, bias, cb):
                # one channel block of the q^T / k^T projection, 4 thunks
                def mk(jt):
                    def thunk():
                        psp = ps_pool.tile([128, 1024], f32, tag="ps")
                        for kk in range(KB):
                            nc.tensor.matmul(
                                psp[:, 0:512],
                                w[:, kk, cb * 128:(cb + 1) * 128],
                                xT_sb[:, kk, jt * 512:(jt + 1) * 512],
                                start=(kk == 0), stop=(kk == KB - 1),
                            )
                        nc.vector.tensor_scalar_add(
                            dst[:, cb, jt * 512:(jt + 1) * 512], psp[:, 0:512],
                            bias[:, cb:cb + 1],
                        )
                    return thunk
                return [mk(jt) for jt in range(JB)]

            def proj_v_thunks():
                # v[t, c] projection, one st-tile per thunk
                def mk(st):
                    def thunk():
                        psp = psC_pool.tile([128, 512], f32, tag="psC")
                        for kk in range(KB):
                            nc.tensor.matmul(
                                psp[:, 0:CS],
                                xT_sb[:, kk, st * 128:(st + 1) * 128],
                                w_vt[:, kk, :],
                                start=(kk == 0), stop=(kk == KB - 1),
                            )
                        nc.vector.tensor_copy(v_sb[:, st, :], psp[:, 0:CS])
                    return thunk
                return [mk(st) for st in range(TB)]

            def load_tail_thunk():
                def thunk():
                    nc.sync.dma_start(
                        out=w_ot, in_=wo_t.ap().rearrange("(k p) m -> p k m", p=128))
                    bo_row = wpool.tile([1, DIM], f32)
                    nc.sync.dma_start(out=bo_row, in_=bo_s.ap().rearrange("(o d) -> o d", o=1))
                    nc.gpsimd.partition_broadcast(bo_sb, bo_row)
                return [thunk]

            def out_proj_thunks(mts):
                # output projection tiles; each needs outT for all 3 pairs in
                # its mt window (i.e. norms (p, mt//4) for all p emitted).
                def mk(mt):
                    def thunk():
                        psD = ps_pool.tile([128, 1024], f32, tag="ps")
                        for ncols in (slice(0, 512), slice(512, DIM)):
                            for cb in range(CB):
                                nc.tensor.matmul(
                                    psD[:, ncols],
                                    outT_sb[:, cb, mt * 128:(mt + 1) * 128],
                                    w_ot[:, cb, ncols],
                                    start=(cb == 0), stop=(cb == CB - 1),
                                )
                        osb = o_pool.tile([128, DIM], f32, tag="osb")
                        nc.vector.tensor_add(osb, psD[:, 0:DIM], bo_sb)
                        nc.sync.dma_start(
                            out=out_p.ap()[mt * 128:(mt + 1) * 128, :], in_=osb,
                        )
                    return thunk
                return [mk(mt) for mt in mts]

            # ---- per-step emitters ----
            e_tiles = {}     # (chunk, s) -> e tile (live for one j-chunk lag)
            psC_cur = {}     # chunk -> psC tile
            acc_cur = {}     # chunk -> bf16 sum accumulator tile

            def emit_scores(c, s):
                pair, j = divmod(c, JB)
                jcols = slice(j * 512, (j + 1) * 512)
                psB = ps_pool.tile([128, 1024], f32, tag="ps")
                nc.tensor.matmul(
                    psB[:, 0:512],
                    kT_sb[0:64, pair, s * 128:(s + 1) * 128],
                    qT_sb[0:64, pair, jcols],
                    start=True, stop=True,
                )
                nc.tensor.matmul(
                    psB[:, 512:1024],
                    kT_sb[64:128, pair, s * 128:(s + 1) * 128],
                    qT_sb[64:128, pair, jcols],
                    start=True, stop=True,
                )
                e = e_pool.tile([128, 1024], bf16, tag="e")
                nc.scalar.activation(
                    e, psB, mybir.ActivationFunctionType.Exp, scale=SCALE,
                )
                e_tiles[(c, s)] = e

            def emit_pv(c, s):
                pair, j = divmod(c, JB)
                if s == 0:
                    psC_cur[c] = psC_pool.tile([128, 512], f32, tag="psC", name=f"psC{c}")
                    acc_cur[c] = e_pool.tile([128, 1024], bf16, tag="acc",
                                             bufs=2, name=f"acc{c}")
                psC = psC_cur[c]
                acc = acc_cur[c]
                e = e_tiles.pop((c, s))
                first, last = (s == 0), (s == TB - 1)
                for hb in range(2):
                    h = 2 * pair + hb
                    ecols = slice(hb * 512, hb * 512 + 512)
                    nc.tensor.matmul(
                        psC[hb * 64:(hb + 1) * 64, :],
                        v_sb[:, s, h * HD:(h + 1) * HD],
                        e[:, ecols],
                        start=first, stop=last,
                    )
                if first:
                    nc.vector.tensor_copy(acc, e)
                else:
                    nc.vector.tensor_add(acc, acc, e)

            def emit_norm(c):
                pair, j = divmod(c, JB)
                jcols = slice(j * 512, (j + 1) * 512)
                psC = psC_cur.pop(c)
                acc = acc_cur.pop(c)
                psS = psS_pool.tile([33, 512], f32, tag="psS")
                nc.tensor.matmul(psS[0:1, :], ones_col, acc[:, 0:512],
                                 start=True, stop=True)
                nc.tensor.matmul(psS[32:33, :], ones_col, acc[:, 512:1024],
                                 start=True, stop=True)
                nr = n_pool.tile([33, 512], f32, tag="nr")
                nc.vector.reciprocal(nr, psS)
                # partition-broadcast of the two recip rows via a DRAM bounce
                # (gpsimd partition_broadcast reads the wrong partition for
                # base!=0 sources on HW)
                rsA = rs_pool.tile([1, 512], f32, tag="rs")
                nc.sync.dma_start(out=rsA, in_=nr[0:1, :])
                rsB = rs_pool.tile([1, 512], f32, tag="rs")
                nc.sync.dma_start(out=rsB, in_=nr[32:33, :])
                nrbc = n_pool.tile([128, 512], f32, tag="nrbc")
                bcA = bass.AP(tensor=rsA.tensor, offset=rsA.offset,
                              ap=[[0, 64], [1, 512]])
                bcB = bass.AP(tensor=rsB.tensor, offset=rsB.offset,
                              ap=[[0, 64], [1, 512]])
                nc.sync.dma_start(out=nrbc[0:64, :], in_=bcA)
                nc.sync.dma_start(out=nrbc[64:128, :], in_=bcB)
                nc.vector.tensor_mul(outT_sb[:, pair, jcols], psC, nrbc)

            # ---- prologue: q/k projections for pair 0 ----
            for t in proj_qk_thunks(kT_sb, w_kt, bk_sb, 0):
                t()
            for t in proj_qk_thunks(qT_sb, w_qt, bq_sb, 0):
                t()

            # ---- injection lists per chunk ----
            inject = {c: [] for c in range(NCHUNK)}
            inject[0] = proj_v_thunks()
            inject[1] = (proj_qk_thunks(kT_sb, w_kt, bk_sb, 1)
                         + proj_qk_thunks(qT_sb, w_qt, bq_sb, 1))
            inject[4] = (proj_qk_thunks(kT_sb, w_kt, bk_sb, 2)
                         + proj_qk_thunks(qT_sb, w_qt, bq_sb, 2)
                         + load_tail_thunk())
            inject[10] = out_proj_thunks(range(0, 4))
            inject[11] = out_proj_thunks(range(4, 8))

            # ---- main loop: scores for chunk c, PV lagged one chunk ----
            for c in range(NCHUNK):
                pending = list(inject[c])
                for s in range(TB):
                    emit_scores(c, s)
                    if pending:
                        pending.pop(0)()
                    if c > 0:
                        emit_pv(c - 1, s)
                if c > 0:
                    emit_norm(c - 1)
                for t in pending:
                    t()

            # ---- tail: last chunk's PV + remaining output projection ----
            tail_inject = out_proj_thunks(range(8, 12))
            for s in range(TB):
                emit_pv(NCHUNK - 1, s)
                if s % 4 == 3:
                    tail_inject.pop(0)()
            emit_norm(NCHUNK - 1)
            for t in out_proj_thunks(range(12, 16)):
                t()

    nc.compile()
    return nc


def _get_program():
    if "nc" not in _PROGRAM_CACHE:
        _PROGRAM_CACHE["nc"] = _build_program()
    return _PROGRAM_CACHE["nc"]


def _prep_in_maps(inputs):
    f = np.float32

    def eff(w, a, bl):
        return (w.astype(np.float64) + bl.astype(np.float64) @ a.astype(np.float64)).astype(f)

    wq = eff(inputs["wq"], inputs["laq"], inputs["lbq"])
    wk = eff(inputs["wk"], inputs["lak"], inputs["lbk"])
    wv = eff(inputs["wv"], inputs["lav"], inputs["lbv"])
    wo = eff(inputs["wo"], inputs["lao"], inputs["lbo"])
    x = np.asarray(inputs["x"], dtype=f)
    bq, bk, bv, bo = (np.asarray(inputs[k], dtype=f) for k in ("bq", "bk", "bv", "bo"))

    in_maps = []
    for core in range(NCORES):
        b, g = core // 2, core % 2
        cols = slice(g * CS, (g + 1) * CS)
        bo_core = wo[:, cols].astype(np.float64) @ bv[cols].astype(np.float64)
        if g == 0:
            bo_core = bo_core + bo
        in_maps.append({
            "xT": _bf16(x[b].T),
            "wq_t": _bf16(wq[cols, :].T),
            "wk_t": _bf16(wk[cols, :].T),
            "wv_t": _bf16(wv[cols, :].T),
            "wo_t": _bf16(wo[:, cols].T),
            "bq_s": np.ascontiguousarray(bq[cols]),
            "bk_s": np.ascontiguousarray(bk[cols]),
            "bo_s": bo_core.astype(f),
        })
    return in_maps


def kernel(**inputs):
    from concourse.bass_utils import run_bass_kernel_spmd

    nc = _get_program()
    in_maps = _prep_in_maps(inputs)
    res = run_bass_kernel_spmd(nc, in_maps, core_ids=list(range(NCORES)))
    out = np.empty((B, T, DIM), dtype=np.float32)
    for b in range(B):
        out[b] = res.results[2 * b]["out_p"] + res.results[2 * b + 1]["out_p"]
    return out
